# revision 1
# baseline (speedup 1.0000x reference)
"""Trainium2 Bass kernel for a 3-layer LSTM discriminator network.

Reference computation (see problem): tracks [256, 512, 2] -> 3-layer LSTM
(H=512, PyTorch gate order i,f,g,o) scanned over T=512 -> ELU(final h of
layer 2) @ W_pred.T + b_pred -> [256, 4].

Strategy
--------
Data-parallel over batch: each of the 8 NeuronCores runs 32 independent
sequences; zero inter-core communication.

Per core, per timestep, each layer needs gates = x @ Wih.T + h @ Whh.T + b
with local batch M=32.  A straight matmul would waste 3/4 of the 128-wide
PE array, so we use 4x column tiling (tile_position=(0, 32j)): the four
column tiles concurrently compute the four 512-column slices of the
(column-permuted) 2048-wide gate matrix, with the transposed activations
[128, 32] as the stationary operand and weight chunks [128, 512] moving.

Gate columns are permuted host-side so PSUM ends up as [128, 512] with
partition p = 32*j + b (stripe j, batch b) and columns [i|f|o|g] for hidden
slice j.  All elementwise LSTM math then runs on full-width [128, N] tiles.
h is transposed back to stationary form each step with 4 col-tiled
matmuls against an identity (no tiling-mode switch, since every matmul in
the program is (128, 32)-tiled).

Biases and the layer-0 input (only 2 features) are folded into one extra
"aug" matmul per layer: stationary = [x0; x1; 1; 0...] (padded to K=128),
moving = [Wih0 rows; bias row; 0...].

Matmuls in bf16 (fp32 runs at 1/4 rate on trn2 PE); cell state c, the
elementwise chain, and the final ELU+head stay fp32.  Verified numerics:
norm rel err ~2e-4 vs fp32 reference.
"""

import sys

if "/opt/trn_rl_repo" not in sys.path:
    sys.path.insert(0, "/opt/trn_rl_repo")

import numpy as np
import ml_dtypes

H = 512
B = 256
T = 512
N_CORES = 8
BL = B // N_CORES  # 32 local batch
NP_ = 4  # NUM_PLAYERS
UNROLL = 8

_CACHE = {}


def _gate_perm():
    # newcol = 512*j + 128*go + c  ->  old gate row
    # stripe-local gate order [i|f|o|g]; PyTorch row order is i,f,g,o.
    base = [0, 512, 1536, 1024]  # i, f, o, g
    perm = np.zeros(4 * H, np.int64)
    n = 0
    for j in range(4):
        for go in range(4):
            for c in range(128):
                perm[n] = base[go] + 128 * j + c
                n += 1
    return perm


def _build_program(t_steps, unroll):
    import concourse.bass as bass
    import concourse.tile as tile
    from concourse import mybir, bacc
    from concourse.bass import ds, ts

    f32 = mybir.dt.float32
    bf16 = mybir.dt.bfloat16
    AF = mybir.ActivationFunctionType

    assert t_steps % unroll == 0

    nc = bacc.Bacc("TRN2", target_bir_lowering=False, num_devices=N_CORES)

    # ---- DRAM parameters ----
    xaug_d = nc.declare_dram_parameter("xaug", [128, t_steps * BL], bf16, isOutput=False)
    w0_d = nc.declare_dram_parameter("w0", [512, 2048], bf16, isOutput=False)
    w0a_d = nc.declare_dram_parameter("w0a", [128, 2048], bf16, isOutput=False)
    w1_d = nc.declare_dram_parameter("w1", [1024, 2048], bf16, isOutput=False)
    w1a_d = nc.declare_dram_parameter("w1a", [128, 2048], bf16, isOutput=False)
    w2_d = nc.declare_dram_parameter("w2", [1024, 2048], bf16, isOutput=False)
    w2a_d = nc.declare_dram_parameter("w2a", [128, 2048], bf16, isOutput=False)
    ones_d = nc.declare_dram_parameter("ones32", [128, 32], bf16, isOutput=False)
    onesf_d = nc.declare_dram_parameter("ones32f", [128, 32], f32, isOutput=False)
    id_d = nc.declare_dram_parameter("ident", [128, 128], bf16, isOutput=False)
    idf_d = nc.declare_dram_parameter("identf", [128, 128], f32, isOutput=False)
    wp_d = nc.declare_dram_parameter("wpred", [512, NP_], f32, isOutput=False)
    bp_d = nc.declare_dram_parameter("bpred", [128, NP_], f32, isOutput=False)
    out_d = nc.declare_dram_parameter("out", [BL, NP_], f32, isOutput=True)

    with tile.TileContext(nc) as tc:
        with (
            tc.tile_pool(name="wpool", bufs=1) as wp,
            tc.tile_pool(name="spool", bufs=1) as sp,
            tc.tile_pool(name="psum", bufs=1, space="PSUM") as pp,
        ):
            # ---- weight tiles ----
            w0t = wp.tile([128, 4 * 2048], bf16, tag="w0t")
            w0at = wp.tile([128, 2048], bf16, tag="w0at")
            w1t = wp.tile([128, 8 * 2048], bf16, tag="w1t")
            w1at = wp.tile([128, 2048], bf16, tag="w1at")
            w2t = wp.tile([128, 8 * 2048], bf16, tag="w2t")
            w2at = wp.tile([128, 2048], bf16, tag="w2at")
            xat = wp.tile([128, t_steps * BL], bf16, tag="xat")
            onest = wp.tile([128, 32], bf16, tag="onest")
            onesft = wp.tile([128, 32], f32, tag="onesft")
            idt = wp.tile([128, 128], bf16, tag="idt")
            idft = wp.tile([128, 128], f32, tag="idft")
            wpt = wp.tile([128, 4 * NP_], f32, tag="wpt")
            bpt = wp.tile([128, NP_], f32, tag="bpt")

            for k in range(4):
                nc.sync.dma_start(w0t[:, ts(k, 2048)], w0_d[128 * k : 128 * (k + 1), :])
            for k in range(8):
                nc.sync.dma_start(w1t[:, ts(k, 2048)], w1_d[128 * k : 128 * (k + 1), :])
                nc.sync.dma_start(w2t[:, ts(k, 2048)], w2_d[128 * k : 128 * (k + 1), :])
            for k in range(4):
                nc.sync.dma_start(wpt[:, ts(k, NP_)], wp_d[128 * k : 128 * (k + 1), :])
            nc.sync.dma_start(w0at[:], w0a_d[:])
            nc.sync.dma_start(w1at[:], w1a_d[:])
            nc.sync.dma_start(w2at[:], w2a_d[:])
            nc.sync.dma_start(xat[:], xaug_d[:])
            nc.sync.dma_start(onest[:], ones_d[:])
            nc.sync.dma_start(onesft[:], onesf_d[:])
            nc.sync.dma_start(idt[:], id_d[:])
            nc.sync.dma_start(idft[:], idf_d[:])
            nc.sync.dma_start(bpt[:], bp_d[:])

            # ---- state tiles ----
            hT = [sp.tile([128, 128], bf16, tag=f"hT{l}", name=f"hT{l}") for l in range(3)]
            hb = [sp.tile([128, 128], bf16, tag=f"hb{l}", name=f"hb{l}") for l in range(3)]
            ct = [sp.tile([128, 128], f32, tag=f"c{l}", name=f"c{l}") for l in range(3)]
            h2f = sp.tile([128, 128], f32, tag="h2f")
            sg = [sp.tile([128, 384], f32, tag=f"sg{l}", name=f"sg{l}") for l in range(3)]
            tg = [sp.tile([128, 128], f32, tag=f"tg{l}", name=f"tg{l}") for l in range(3)]
            tcl = [sp.tile([128, 128], f32, tag=f"tc{l}", name=f"tc{l}") for l in range(3)]
            m2 = [sp.tile([128, 128], f32, tag=f"m2{l}", name=f"m2{l}") for l in range(3)]

            for l in range(3):
                nc.gpsimd.memset(hT[l][:], 0.0)
                nc.gpsimd.memset(hb[l][:], 0.0)
                nc.gpsimd.memset(ct[l][:], 0.0)

            # ---- psum tiles ----
            gps = [pp.tile([128, 512], f32, tag=f"g{l}", name=f"g{l}") for l in range(3)]
            pts = [pp.tile([128, 128], f32, tag=f"pt{l}", name=f"pt{l}") for l in range(3)]
            phead = pp.tile([32, NP_], f32, tag="phead")

            # current-step x slice staged to a fixed address (ldweights cannot
            # take register offsets); two buffers so the copy prefetches.
            xcur = [
                sp.tile([128, 32], bf16, tag=f"xcur{i}", name=f"xcur{i}")
                for i in range(2)
            ]

            wts = [w0t, w1t, w2t]
            wats = [w0at, w1at, w2at]

            def mm_rounds(l, chunks, first, last):
                """Issue col-tiled matmul rounds for layer l.

                chunks: list of (stationary_ap_fn(j->AP is same for all j), moving_col_base)
                """
                g = gps[l]
                n = len(chunks)
                for idx, (stat, movt, mcol) in enumerate(chunks):
                    st = first and idx == 0
                    sp_ = last and idx == n - 1
                    for j in range(4):
                        nc.tensor.matmul(
                            g[32 * j : 32 * (j + 1), :],
                            stat,
                            movt[:, mcol + 512 * j : mcol + 512 * (j + 1)],
                            start=st,
                            stop=sp_,
                            skip_group_check=True,
                            tile_position=(0, 32 * j),
                        )

            def transpose_h(l):
                # hb[l] [128(j,b), 128(c)] -> hT[l] [128(c), 128(j,b)]
                for j in range(4):
                    nc.tensor.matmul(
                        pts[l][32 * j : 32 * (j + 1), :],
                        hb[l][:, 32 * j : 32 * (j + 1)],
                        idt[:],
                        start=True,
                        stop=True,
                        skip_group_check=True,
                        tile_position=(0, 32 * j),
                    )
                nc.scalar.activation(hT[l][:], pts[l][:], AF.Copy)

            def elem(l):
                g = gps[l]
                nc.scalar.activation(sg[l][:, 0:384], g[:, 0:384], AF.Sigmoid)
                nc.scalar.activation(tg[l][:], g[:, 384:512], AF.Tanh)
                nc.vector.tensor_mul(ct[l][:], sg[l][:, 128:256], ct[l][:])
                nc.vector.tensor_mul(m2[l][:], sg[l][:, 0:128], tg[l][:])
                nc.vector.tensor_add(ct[l][:], ct[l][:], m2[l][:])
                nc.scalar.activation(tcl[l][:], ct[l][:], AF.Tanh)
                nc.vector.tensor_mul(hb[l][:], sg[l][:, 256:384], tcl[l][:])
                if l == 2:
                    nc.vector.tensor_mul(h2f[:], sg[l][:, 256:384], tcl[l][:])

            def l0_mms(toff, u):
                xc = xcur[u % 2]
                nc.gpsimd.tensor_copy(xc[:], xat[:, ds(toff, 32)])
                # aug first (always ready), then 4 h-chunks
                chunks = [(xc[:], w0at, 0)]
                for k in range(4):
                    chunks.append((hT[0][:, 32 * k : 32 * (k + 1)], w0t, k * 2048))
                mm_rounds(0, chunks, True, True)

            def l_augh_mms(l):
                # aug + own-h chunks (depend only on previous step)
                chunks = [(onest[:, 0:32], wats[l], 0)]
                for k in range(4):
                    chunks.append((hT[l][:, 32 * k : 32 * (k + 1)], wts[l], (4 + k) * 2048))
                mm_rounds(l, chunks, True, False)

            def l_x_mms(l):
                # x chunks: depend on this step's h of layer l-1
                chunks = []
                for k in range(4):
                    chunks.append((hT[l - 1][:, 32 * k : 32 * (k + 1)], wts[l], k * 2048))
                mm_rounds(l, chunks, False, True)

            def step(toff, u):
                # software-pipelined issue order; T2 of previous step is issued
                # after l0's matmuls so PE has fill work during elem2(prev).
                l0_mms(toff, u)
                transpose_h(2)  # prev step's h2 (first iter: hb2 is zeros)
                elem(0)
                l_augh_mms(1)
                transpose_h(0)
                l_x_mms(1)
                elem(1)
                l_augh_mms(2)
                transpose_h(1)
                l_x_mms(2)
                elem(2)

            with tc.For_i(0, t_steps * BL, BL * unroll) as toff:
                for u in range(unroll):
                    step(toff + BL * u, u)

            # ---- final head: ELU(h2) @ W_pred.T + b_pred ----
            hp = sp.tile([128, 128], f32, tag="hp")
            hn = sp.tile([128, 128], f32, tag="hn")
            eh = sp.tile([128, 128], f32, tag="eh")
            ehT = sp.tile([128, 128], f32, tag="ehT")
            outs = sp.tile([32, NP_], f32, tag="outs")
            pth = pp.tile([128, 128], f32, tag="pth")

            nc.vector.tensor_scalar_max(hp[:], h2f[:], 0.0)
            nc.vector.tensor_scalar_min(hn[:], h2f[:], 0.0)
            nc.scalar.activation(hn[:], hn[:], AF.Exp)
            nc.vector.tensor_add(eh[:], hp[:], hn[:])
            nc.vector.tensor_scalar_sub(eh[:], eh[:], 1.0)
            for j in range(4):
                nc.tensor.matmul(
                    pth[32 * j : 32 * (j + 1), :],
                    eh[:, 32 * j : 32 * (j + 1)],
                    idft[:],
                    start=True,
                    stop=True,
                    skip_group_check=True,
                    tile_position=(0, 32 * j),
                )
            nc.scalar.activation(ehT[:], pth[:], AF.Copy)
            for k in range(4):
                nc.tensor.matmul(
                    phead[:, :],
                    ehT[:, 32 * k : 32 * (k + 1)],
                    wpt[:, NP_ * k : NP_ * (k + 1)],
                    start=(k == 0),
                    stop=False,
                    skip_group_check=True,
                    tile_position=(0, 0),
                )
            nc.tensor.matmul(
                phead[:, :], onesft[:, 0:32], bpt[:], start=False, stop=True,
                skip_group_check=True, tile_position=(0, 0),
            )
            nc.scalar.activation(outs[:], phead[:, :], AF.Copy)
            nc.sync.dma_start(out_d[:], outs[:])

    nc.compile()
    return nc


def _prep_inputs(tracks, weights, t_steps):
    """Build per-core input maps. weights: dict of the 14 weight arrays."""
    bf = ml_dtypes.bfloat16
    perm = _gate_perm()

    def pw(a):  # permute gate columns of a [*, 2048] matrix
        return np.ascontiguousarray(a[:, perm])

    W = {k: np.asarray(v, np.float32) for k, v in weights.items()}

    w0 = pw(W["W_hh0"].T).astype(bf)
    w0a = np.zeros((128, 2048), bf)
    w0a[0:2] = pw(W["W_ih0"].T).astype(bf)
    w0a[2] = (W["b_ih0"] + W["b_hh0"])[perm].astype(bf)

    def wl(l):
        wm = np.vstack([pw(W[f"W_ih{l}"].T), pw(W[f"W_hh{l}"].T)]).astype(bf)
        wa = np.zeros((128, 2048), bf)
        wa[0] = (W[f"b_ih{l}"] + W[f"b_hh{l}"])[perm].astype(bf)
        return wm, wa

    w1, w1a = wl(1)
    w2, w2a = wl(2)

    ones32 = np.zeros((128, 32), bf)
    ones32[0] = 1
    ones32f = np.zeros((128, 32), np.float32)
    ones32f[0] = 1
    ident = np.eye(128, dtype=bf)
    identf = np.eye(128, dtype=np.float32)
    wpred = np.ascontiguousarray(W["W_pred"].T.astype(np.float32))
    bpred = np.zeros((128, NP_), np.float32)
    bpred[0] = W["b_pred"]

    shared = dict(
        w0=w0, w0a=w0a, w1=w1, w1a=w1a, w2=w2, w2a=w2a,
        ones32=ones32, ones32f=ones32f, ident=ident, identf=identf,
        wpred=wpred, bpred=bpred,
    )

    tracks = np.asarray(tracks, np.float32)
    in_maps = []
    for c in range(N_CORES):
        tc_ = tracks[c * BL : (c + 1) * BL, :t_steps]  # [BL, t, 2]
        xa = np.zeros((128, t_steps * BL), bf)
        xa[0] = tc_[:, :, 0].T.reshape(-1).astype(bf)
        xa[1] = tc_[:, :, 1].T.reshape(-1).astype(bf)
        xa[2] = 1
        m = dict(shared)
        m["xaug"] = xa
        in_maps.append(m)
    return in_maps


def _get_program(t_steps, unroll):
    key = (t_steps, unroll)
    if key not in _CACHE:
        _CACHE[key] = _build_program(t_steps, unroll)
    return _CACHE[key]


def kernel(**inputs):
    from concourse.bass_utils import run_bass_kernel_spmd

    tracks = np.asarray(inputs["tracks"])
    weights = {k: v for k, v in inputs.items() if k != "tracks"}
    nc = _get_program(T, UNROLL)
    in_maps = _prep_inputs(tracks, weights, T)
    res = run_bass_kernel_spmd(nc, in_maps, list(range(N_CORES)))
    out = np.concatenate([res.results[c]["out"] for c in range(N_CORES)], axis=0)
    return out.astype(np.float32)

